# revision 17
# baseline (speedup 1.0000x reference)
"""Trainium2 Bass kernel: attention layer with KV cache, tensor-parallel over heads.

Sharding (8 NeuronCores): Megatron-style TP over the 32 heads -> 4 heads/core.
  - wq/wk/wv: column-parallel (each core owns a [512, 4096] output shard)
  - wo: row-parallel (each core owns wo[:, c*512:(c+1)*512]); cores emit
    partial o-proj outputs which the host sums (RowParallel unshard).
  - cache_k/cache_v: sharded along the head axis; history rows/positions are
    gathered host-side from batch_exec/start_pos (pure indexing).

Device layout trick: Q/K are projected directly in transposed [head_dim, tok]
layout (head dim = PSUM partitions), V in natural [tok, head_dim] layout, so
scores^T, PV, and the o-projection all consume each other's outputs as
matmul operands with zero on-device transposes.  Softmax runs without the
max-subtraction (scores are O(10), exp is safe in f32) so the kv-axis
(partition-axis) row-sum comes from a ones-vector matmul; normalization is a
rank-1 broadcast matmul of 1/r.
"""

import numpy as np
import ml_dtypes

import concourse.bass as bass
import concourse.bacc as bacc
import concourse.tile as tile
from concourse import mybir
from concourse.bass_utils import run_bass_kernel_spmd

BF16 = np.dtype(ml_dtypes.bfloat16)

# Problem shape (hardcoded per the task contract)
BSZ = 8
SEQ = 512
DIM = 4096
NH = 32
HD = 128
START = 512
KV = START + SEQ          # 1024
NC = 8                    # cores
HPC = NH // NC            # 4 heads per core
HF = HPC * HD             # 512 local features
P = 128
KC = DIM // P             # 32 contraction chunks
TCH = KV // P             # 8 kv chunks
SC = SEQ // P             # 4 seq chunks
ROPE_BASE = 10000.0

FP32 = mybir.dt.float32
BF16D = mybir.dt.bfloat16


def build_program():
    nc = bacc.Bacc(None, target_bir_lowering=False)
    xT_d = nc.declare_dram_parameter("xT", [DIM, BSZ * SEQ], BF16D, isOutput=False)
    wqkvT_d = nc.declare_dram_parameter("wqkvT", [DIM, 3 * HF], BF16D, isOutput=False)
    woT_d = nc.declare_dram_parameter("woT", [HF, DIM], BF16D, isOutput=False)
    rope_d = nc.declare_dram_parameter("rope", [4, HD, BSZ * SEQ], FP32, isOutput=False)
    kTh_d = nc.declare_dram_parameter("kTh", [BSZ, HPC, HD, START], BF16D, isOutput=False)
    vh_d = nc.declare_dram_parameter("vh", [BSZ, HPC, START, HD], BF16D, isOutput=False)
    gateT_d = nc.declare_dram_parameter("gateT", [BSZ, KV, SEQ], BF16D, isOutput=False)
    out_d = nc.declare_dram_parameter("out", [BSZ * SEQ, DIM], BF16D, isOutput=True)

    xT_r = xT_d.rearrange("(c p) t -> p c t", p=P)          # [128, 32, 4096]
    wqkvT_r = wqkvT_d.rearrange("(c p) f -> p c f", p=P)    # [128, 32, 1536]
    woT_r = woT_d.rearrange("(h p) o -> p h o", p=P)        # [128, 4, 4096]

    from contextlib import ExitStack

    with ExitStack() as ctx:
        tc = ctx.enter_context(tile.TileContext(nc))
        cpool = ctx.enter_context(tc.tile_pool(name="const", bufs=1))
        wupool = ctx.enter_context(tc.tile_pool(name="wu", bufs=3))
        xpool = ctx.enter_context(tc.tile_pool(name="xb", bufs=1))
        rpool = ctx.enter_context(tc.tile_pool(name="rope", bufs=1))
        qkvpool = ctx.enter_context(tc.tile_pool(name="qkv", bufs=2))
        hpool = ctx.enter_context(tc.tile_pool(name="hist", bufs=1))
        wkpool = ctx.enter_context(tc.tile_pool(name="work", bufs=2))
        epool = ctx.enter_context(tc.tile_pool(name="ee", bufs=1))
        apool = ctx.enter_context(tc.tile_pool(name="at", bufs=1))
        smpool = ctx.enter_context(tc.tile_pool(name="small", bufs=1))
        gpool = ctx.enter_context(tc.tile_pool(name="gate", bufs=2))
        popool = ctx.enter_context(tc.tile_pool(name="po", bufs=2))
        pA = ctx.enter_context(tc.tile_pool(name="pA", bufs=2, space="PSUM"))
        pS = ctx.enter_context(tc.tile_pool(name="pS", bufs=2, space="PSUM"))
        pR = ctx.enter_context(tc.tile_pool(name="pR", bufs=1, space="PSUM"))
        pRB = ctx.enter_context(tc.tile_pool(name="pRB", bufs=1, space="PSUM"))
        pOT = ctx.enter_context(tc.tile_pool(name="pOT", bufs=1, space="PSUM"))
        pP = ctx.enter_context(tc.tile_pool(name="pP", bufs=1, space="PSUM"))
        if True:
            # ---- constants (weights emitted after b0's hot DMAs, see below) ----
            ones_bf = cpool.tile([P, 1], BF16D)
            nc.gpsimd.memset(ones_bf[:], 1.0)
            ones_f1 = cpool.tile([1, P], FP32)
            nc.gpsimd.memset(ones_f1[:], 1.0)
            wv_s = cpool.tile([P, KC, HF], BF16D)
            woT_s = cpool.tile([P, HPC, DIM], BF16D)

            for b in range(BSZ):
                ts, te = b * SEQ, (b + 1) * SEQ
                xb = xpool.tile([P, KC, SEQ], BF16D)
                for xc in range(4):  # split so the first matmuls start early
                    nc.sync.dma_start(
                        xb[:, xc * 8:(xc + 1) * 8, :],
                        xT_r[:, xc * 8:(xc + 1) * 8, ts:te],
                    )
                rope_b = rpool.tile([P, 4, SEQ], FP32)
                nc.sync.dma_start(rope_b[:], rope_d.rearrange("r p t -> p r t")[:, :, ts:te])
                if b == 0:
                    # v weights land behind b0's q/k units; wo later still
                    nc.sync.dma_start(wv_s[:, 0:16, :], wqkvT_r[:, 0:16, 2 * HF:3 * HF])
                    nc.sync.dma_start(wv_s[:, 16:32, :], wqkvT_r[:, 16:32, 2 * HF:3 * HF])
                    nc.sync.dma_start(woT_s[:, 0:2, :], woT_r[:, 0:2, :])
                    nc.sync.dma_start(woT_s[:, 2:4, :], woT_r[:, 2:4, :])
                kThb = hpool.tile([P, HPC, START], BF16D)
                nc.sync.dma_start(kThb[:], kTh_d[b].rearrange("h p t -> p h t"))
                vhb = hpool.tile([P, HPC * SC, HD], BF16D)
                nc.sync.dma_start(vhb[:], vh_d[b].rearrange("h (c p) d -> p (h c) d", p=P))
                gtb = gpool.tile([P, TCH, SEQ], BF16D)
                nc.sync.dma_start(gtb[:], gateT_d[b].rearrange("(c p) s -> p c s", p=P))

                # ---- phase A: QKV projections (+RoPE for q/k) ----
                qT_b = qkvpool.tile([P, HPC, SEQ], BF16D, tag="qT")
                kT_b = qkvpool.tile([P, HPC, SEQ], BF16D, tag="kT")
                v_b = qkvpool.tile([P, HPC, SEQ], BF16D, tag="v")

                for proj in range(2):  # 0=q, 1=k
                    dst = qT_b if proj == 0 else kT_b
                    ct, st = (0, 1) if proj == 0 else (2, 3)
                    for h in range(HPC):
                        wu = wupool.tile([P, KC, P], BF16D)
                        nc.sync.dma_start(
                            wu[:], wqkvT_r[:, :, proj * HF + h * P: proj * HF + (h + 1) * P]
                        )
                        ps = pA.tile([P, SEQ], FP32, tag="pa")
                        for kc in range(KC):
                            nc.tensor.matmul(
                                ps[:], wu[:, kc, :], xb[:, kc, :],
                                start=(kc == 0), stop=(kc == KC - 1),
                            )
                        # RoPE: dst = ps*cos + shift64(ps)*sin_signed
                        t1 = wkpool.tile([P, SEQ], FP32, tag="t1")
                        nc.vector.tensor_mul(t1[:], ps[:], rope_b[:, ct, :])
                        t2 = wkpool.tile([P, SEQ], FP32, tag="t2")
                        H2 = HD // 2
                        nc.vector.tensor_mul(t2[0:H2, :], ps[H2:P, :], rope_b[0:H2, st, :])
                        nc.vector.tensor_mul(t2[H2:P, :], ps[0:H2, :], rope_b[H2:P, st, :])
                        nc.vector.tensor_add(dst[:, h, :], t1[:], t2[:])

                for st4 in range(SC):  # v, natural layout
                    ps = pA.tile([P, HF], FP32, tag="pa")
                    for kc in range(KC):
                        nc.tensor.matmul(
                            ps[:], xb[:, kc, st4 * P:(st4 + 1) * P], wv_s[:, kc, :],
                            start=(kc == 0), stop=(kc == KC - 1),
                        )
                    nc.scalar.activation(v_b[:, st4, :], ps[:], mybir.ActivationFunctionType.Copy)

                # ---- phase B: attention per head ----
                # Causal structure: kv chunk t8 >= SC only reaches queries
                # s >= (t8-SC)*P; the rest is masked, so narrow every op.
                aT = apool.tile([P, HPC, SEQ], BF16D)
                for h in range(HPC):
                    ee = epool.tile([P, TCH, SEQ], BF16D)
                    for t8 in range(TCH):
                        s0 = 0 if t8 < SC else (t8 - SC) * P
                        if t8 < SC:
                            klhs = kThb[:, h, t8 * P:(t8 + 1) * P]
                        else:
                            klhs = kT_b[:, h, (t8 - SC) * P:(t8 - SC + 1) * P]
                        pscr = pS.tile([P, SEQ], FP32, tag="ps")
                        nc.tensor.matmul(pscr[:, s0:], klhs, qT_b[:, h, s0:], start=True, stop=True)
                        tmpe = wkpool.tile([P, SEQ], BF16D, tag="te")
                        nc.scalar.activation(tmpe[:, s0:], pscr[:, s0:], mybir.ActivationFunctionType.Exp)
                        nc.vector.tensor_mul(ee[:, t8, s0:], tmpe[:, s0:], gtb[:, t8, s0:])
                    # row-sum over kv (partition axis) via ones-matmul
                    pr = pR.tile([1, SEQ], FP32, tag="pr")
                    for t8 in range(TCH):
                        s0 = 0 if t8 < SC else (t8 - SC) * P
                        nc.tensor.matmul(
                            pr[:, s0:], ones_bf[:], ee[:, t8, s0:],
                            start=(t8 == 0), stop=(t8 == TCH - 1),
                            skip_group_check=True,
                        )
                    rinv = smpool.tile([1, SEQ], FP32, tag="rinv")
                    nc.vector.reciprocal(rinv[:], pr[:])
                    prb = pRB.tile([P, SEQ], FP32, tag="prb")
                    nc.tensor.matmul(prb[:], ones_f1[:], rinv[:], start=True, stop=True)
                    rb_s = smpool.tile([P, SEQ], FP32, tag="rbs")
                    nc.scalar.activation(rb_s[:], prb[:], mybir.ActivationFunctionType.Copy)
                    # oT = V^T E (accumulate over kv chunks)
                    po = pOT.tile([P, SEQ], FP32, tag="po")
                    for t8 in range(TCH):
                        s0 = 0 if t8 < SC else (t8 - SC) * P
                        if t8 < SC:
                            vlhs = vhb[:, h * SC + t8, :]
                        else:
                            vlhs = v_b[:, t8 - SC, h * P:(h + 1) * P]
                        nc.tensor.matmul(
                            po[:, s0:], vlhs, ee[:, t8, s0:],
                            start=(t8 == 0), stop=(t8 == TCH - 1),
                            skip_group_check=True,
                        )
                    nc.vector.tensor_mul(aT[:, h, :], po[:], rb_s[:])

                # ---- o-projection (partial, row-parallel) ----
                for sc4 in range(SC):
                    for oc in range(TCH):
                        pp = pP.tile([P, HF], FP32, tag="pp")
                        for h in range(HPC):
                            nc.tensor.matmul(
                                pp[:],
                                aT[:, h, sc4 * P:(sc4 + 1) * P],
                                woT_s[:, h, oc * HF:(oc + 1) * HF],
                                start=(h == 0), stop=(h == HPC - 1),
                            )
                        pout = popool.tile([P, HF], BF16D, tag="pout")
                        nc.scalar.activation(pout[:], pp[:], mybir.ActivationFunctionType.Copy)
                        nc.sync.dma_start(
                            out_d[ts + sc4 * P: ts + (sc4 + 1) * P, oc * HF:(oc + 1) * HF],
                            pout[:],
                        )
    nc.finalize()
    return nc


_CACHE = {}


def _get_program():
    if "nc" not in _CACHE:
        _CACHE["nc"] = build_program()
    return _CACHE["nc"]


def _prep_inputs(inputs):
    x = np.asarray(inputs["x"], np.float32)
    router = np.asarray(inputs["router"], np.float32)
    cache_k = np.asarray(inputs["cache_k"], np.float32)
    cache_v = np.asarray(inputs["cache_v"], np.float32)
    cache_mask = np.asarray(inputs["cache_mask"])
    mask = np.asarray(inputs["mask"], np.float32)
    wq = np.asarray(inputs["wq"], np.float32)
    wk = np.asarray(inputs["wk"], np.float32)
    wv = np.asarray(inputs["wv"], np.float32)
    wo = np.asarray(inputs["wo"], np.float32)
    position_ids = np.asarray(inputs["position_ids"], np.int64)
    batch_exec = np.asarray(inputs["batch_exec"], np.int64)
    start_pos = int(inputs["start_pos"])
    assert start_pos == START and x.shape == (BSZ, SEQ, DIM)

    xT = np.ascontiguousarray(x.reshape(BSZ * SEQ, DIM).T).astype(BF16)

    # RoPE tables gathered at position_ids, in [hd, tok] layout
    inv_freq = 1.0 / (ROPE_BASE ** (np.arange(0, HD, 2, dtype=np.float32) / HD))
    t = np.arange(KV, dtype=np.float32)
    emb = np.concatenate([t[:, None] * inv_freq, t[:, None] * inv_freq], axis=-1)
    cos_t = np.cos(emb).astype(np.float32)[position_ids].reshape(BSZ * SEQ, HD)
    sin_t = np.sin(emb).astype(np.float32)[position_ids].reshape(BSZ * SEQ, HD)
    sign = np.where(np.arange(HD) < HD // 2, -1.0, 1.0).astype(np.float32)
    scale = np.float32(1.0 / np.sqrt(HD))
    rope = np.stack([
        cos_t.T * scale,                # cq (q, score-scale folded)
        (sin_t * sign).T * scale,       # sq signed
        cos_t.T,                        # ck
        (sin_t * sign).T,               # sk signed
    ]).astype(np.float32)               # [4, 128, 4096]

    # history cache slices (host-side gather = sharding)
    k_hist = cache_k[batch_exec, :, :START, :]   # [8, 32, 512, 128]
    v_hist = cache_v[batch_exec, :, :START, :]

    # multiplicative 0/1 gate, transposed [b, t, s]: causal AND cache-usable
    pen_hist = cache_mask[batch_exec, :START].astype(bool)            # [8, 512]
    pen_new = (router[:, :, 0] != 0.0)                                # [8, 512]
    pen = np.concatenate([pen_hist, pen_new], axis=1)                 # [8, 1024]
    causal_ok = (mask[0, 0] > -0.5)                                   # [512 s, 1024 t]
    gate = causal_ok.T[None, :, :] & pen[:, :, None]                  # [8, 1024, 512]
    gateT = np.ascontiguousarray(gate.astype(np.float32)).astype(BF16)

    in_maps = []
    for c in range(NC):
        hs, he = c * HPC, (c + 1) * HPC
        fs, fe = c * HF, (c + 1) * HF
        wqkvT = np.concatenate(
            [wq[fs:fe].T, wk[fs:fe].T, wv[fs:fe].T], axis=1).astype(BF16)
        woT = np.ascontiguousarray(wo[:, fs:fe].T).astype(BF16)
        kTh = np.ascontiguousarray(
            k_hist[:, hs:he].transpose(0, 1, 3, 2)).astype(BF16)
        vh = np.ascontiguousarray(v_hist[:, hs:he]).astype(BF16)
        in_maps.append({
            "xT": xT,
            "wqkvT": np.ascontiguousarray(wqkvT),
            "woT": woT,
            "rope": rope,
            "kTh": kTh,
            "vh": vh,
            "gateT": gateT,
        })
    return in_maps


def _install_profile_hook():
    """The agent image's antenv lacks axon_hooks; shim it so trace=True works."""
    import sys, types
    if "antenv.axon_hooks" in sys.modules:
        return
    try:
        from trn_agent_boot.trn_boot import _ntff_profile_via_ctypes
    except ImportError:
        return
    mod = types.ModuleType("antenv.axon_hooks")
    mod._hook = _ntff_profile_via_ctypes("/opt/axon/libaxon_pjrt.so")

    def set_axon_ntff_profile_hook(h):
        mod._hook = h

    def get_axon_ntff_profile_hook():
        return mod._hook

    mod.set_axon_ntff_profile_hook = set_axon_ntff_profile_hook
    mod.get_axon_ntff_profile_hook = get_axon_ntff_profile_hook
    sys.modules["antenv.axon_hooks"] = mod
    import antenv
    antenv.axon_hooks = mod


def _run(inputs, trace=False):
    if trace:
        _install_profile_hook()
    nc = _get_program()
    in_maps = _prep_inputs(inputs)
    res = run_bass_kernel_spmd(nc, in_maps, core_ids=list(range(NC)), trace=trace)
    acc = np.zeros((BSZ * SEQ, DIM), np.float32)
    for c in range(NC):
        acc += res.results[c]["out"].astype(np.float32)
    return acc.reshape(BSZ, SEQ, DIM), res


def kernel(**inputs):
    out, _ = _run(inputs, trace=False)
    return out


# revision 21
# speedup vs baseline: 1.2162x; 1.2162x over previous
"""Trainium2 Bass kernel: attention layer with KV cache, tensor-parallel over heads.

Sharding (8 NeuronCores): Megatron-style TP over the 32 heads -> 4 heads/core.
  - wq/wk/wv: column-parallel (each core owns a [512, 4096] output shard)
  - wo: row-parallel (each core owns wo[:, c*512:(c+1)*512]); cores emit
    partial o-proj outputs which the host sums (RowParallel unshard).
  - cache_k/cache_v: sharded along the head axis; history rows/positions are
    gathered host-side from batch_exec/start_pos (pure indexing).

Device layout trick: Q/K are projected directly in transposed [head_dim, tok]
layout (head dim = PSUM partitions), V in natural [tok, head_dim] layout, so
scores^T, PV, and the o-projection all consume each other's outputs as
matmul operands with zero on-device transposes.  Softmax runs without the
max-subtraction (scores are O(10), exp is safe in f32) so the kv-axis
(partition-axis) row-sum comes from a ones-vector matmul; normalization is a
rank-1 broadcast matmul of 1/r.
"""

import numpy as np
import ml_dtypes

import concourse.bass as bass
import concourse.bacc as bacc
import concourse.tile as tile
from concourse import mybir
from concourse.bass_utils import run_bass_kernel_spmd

BF16 = np.dtype(ml_dtypes.bfloat16)

# Problem shape (hardcoded per the task contract)
BSZ = 8
SEQ = 512
DIM = 4096
NH = 32
HD = 128
START = 512
KV = START + SEQ          # 1024
NC = 8                    # cores
HPC = NH // NC            # 4 heads per core
HF = HPC * HD             # 512 local features
P = 128
KC = DIM // P             # 32 contraction chunks
TCH = KV // P             # 8 kv chunks
SC = SEQ // P             # 4 seq chunks
ROPE_BASE = 10000.0

FP32 = mybir.dt.float32
BF16D = mybir.dt.bfloat16


def build_program():
    nc = bacc.Bacc(None, target_bir_lowering=False)
    xT_d = nc.declare_dram_parameter("xT", [DIM, BSZ * SEQ], BF16D, isOutput=False)
    wqkvT_d = nc.declare_dram_parameter("wqkvT", [DIM, 3 * HF], BF16D, isOutput=False)
    woT_d = nc.declare_dram_parameter("woT", [HF, DIM], BF16D, isOutput=False)
    rope_d = nc.declare_dram_parameter("rope", [4, HD, BSZ * SEQ], FP32, isOutput=False)
    kTh_d = nc.declare_dram_parameter("kTh", [BSZ, HPC, HD, START], BF16D, isOutput=False)
    vh_d = nc.declare_dram_parameter("vh", [BSZ, HPC, START, HD], BF16D, isOutput=False)
    gateT_d = nc.declare_dram_parameter("gateT", [BSZ, KV, SEQ], BF16D, isOutput=False)
    out_d = nc.declare_dram_parameter("out", [BSZ * SEQ, DIM], BF16D, isOutput=True)

    xT_r = xT_d.rearrange("(c p) t -> p c t", p=P)          # [128, 32, 4096]
    wqkvT_r = wqkvT_d.rearrange("(c p) f -> p c f", p=P)    # [128, 32, 1536]
    woT_r = woT_d.rearrange("(h p) o -> p h o", p=P)        # [128, 4, 4096]

    from contextlib import ExitStack

    with ExitStack() as ctx:
        tc = ctx.enter_context(tile.TileContext(nc))
        cpool = ctx.enter_context(tc.tile_pool(name="const", bufs=1))
        wupool = ctx.enter_context(tc.tile_pool(name="wu", bufs=3))
        xpool = ctx.enter_context(tc.tile_pool(name="xb", bufs=1))
        rpool = ctx.enter_context(tc.tile_pool(name="rope", bufs=1))
        qkvpool = ctx.enter_context(tc.tile_pool(name="qkv", bufs=2))
        hpool = ctx.enter_context(tc.tile_pool(name="hist", bufs=1))
        wkpool = ctx.enter_context(tc.tile_pool(name="work", bufs=2))
        epool = ctx.enter_context(tc.tile_pool(name="ee", bufs=1))
        apool = ctx.enter_context(tc.tile_pool(name="at", bufs=1))
        smpool = ctx.enter_context(tc.tile_pool(name="small", bufs=1))
        gpool = ctx.enter_context(tc.tile_pool(name="gate", bufs=2))
        popool = ctx.enter_context(tc.tile_pool(name="po", bufs=2))
        pA = ctx.enter_context(tc.tile_pool(name="pA", bufs=2, space="PSUM"))
        pS = ctx.enter_context(tc.tile_pool(name="pS", bufs=2, space="PSUM"))
        pR = ctx.enter_context(tc.tile_pool(name="pR", bufs=1, space="PSUM"))
        pOT = ctx.enter_context(tc.tile_pool(name="pOT", bufs=1, space="PSUM"))
        pP = ctx.enter_context(tc.tile_pool(name="pP", bufs=2, space="PSUM"))
        if True:
            # ---- constants (weights emitted after b0's hot DMAs, see below) ----
            ones_bf = cpool.tile([P, 1], BF16D)
            nc.gpsimd.memset(ones_bf[:], 1.0)
            wv_s = cpool.tile([P, KC, HF], BF16D)
            woT_s = cpool.tile([P, HPC, DIM], BF16D)

            for b in range(BSZ):
                ts, te = b * SEQ, (b + 1) * SEQ
                xb = xpool.tile([P, KC, SEQ], BF16D)
                for xc in range(4):  # split so the first matmuls start early
                    nc.sync.dma_start(
                        xb[:, xc * 8:(xc + 1) * 8, :],
                        xT_r[:, xc * 8:(xc + 1) * 8, ts:te],
                    )
                rope_b = rpool.tile([P, 4, SEQ], FP32)
                nc.sync.dma_start(rope_b[:], rope_d.rearrange("r p t -> p r t")[:, :, ts:te])
                if b == 0:
                    # v weights land behind b0's q/k units; wo later still
                    nc.sync.dma_start(wv_s[:, 0:16, :], wqkvT_r[:, 0:16, 2 * HF:3 * HF])
                    nc.sync.dma_start(wv_s[:, 16:32, :], wqkvT_r[:, 16:32, 2 * HF:3 * HF])
                    nc.sync.dma_start(woT_s[:, 0:2, :], woT_r[:, 0:2, :])
                    nc.sync.dma_start(woT_s[:, 2:4, :], woT_r[:, 2:4, :])
                kThb = hpool.tile([P, HPC, START], BF16D)
                nc.sync.dma_start(kThb[:], kTh_d[b].rearrange("h p t -> p h t"))
                vhb = hpool.tile([P, HPC * SC, HD], BF16D)
                nc.sync.dma_start(vhb[:], vh_d[b].rearrange("h (c p) d -> p (h c) d", p=P))
                gtb = gpool.tile([P, TCH, SEQ], BF16D)
                nc.sync.dma_start(gtb[:], gateT_d[b].rearrange("(c p) s -> p c s", p=P))

                # ---- phase A: QKV projections (+RoPE for q/k) ----
                qT_b = qkvpool.tile([P, HPC, SEQ], BF16D, tag="qT")
                kT_b = qkvpool.tile([P, HPC, SEQ], BF16D, tag="kT")
                v_b = qkvpool.tile([P, HPC, SEQ], BF16D, tag="v")

                for proj in range(2):  # 0=q, 1=k
                    dst = qT_b if proj == 0 else kT_b
                    ct, st = (0, 1) if proj == 0 else (2, 3)
                    for h in range(HPC):
                        wu = wupool.tile([P, KC, P], BF16D)
                        nc.sync.dma_start(
                            wu[:], wqkvT_r[:, :, proj * HF + h * P: proj * HF + (h + 1) * P]
                        )
                        ps = pA.tile([P, SEQ], FP32, tag="pa")
                        for kc in range(KC):
                            nc.tensor.matmul(
                                ps[:], wu[:, kc, :], xb[:, kc, :],
                                start=(kc == 0), stop=(kc == KC - 1),
                            )
                        # RoPE: dst = ps*cos + shift64(ps)*sin_signed
                        t1 = wkpool.tile([P, SEQ], FP32, tag="t1")
                        nc.vector.tensor_mul(t1[:], ps[:], rope_b[:, ct, :])
                        t2 = wkpool.tile([P, SEQ], FP32, tag="t2")
                        H2 = HD // 2
                        nc.vector.tensor_mul(t2[0:H2, :], ps[H2:P, :], rope_b[0:H2, st, :])
                        nc.vector.tensor_mul(t2[H2:P, :], ps[0:H2, :], rope_b[H2:P, st, :])
                        nc.vector.tensor_add(dst[:, h, :], t1[:], t2[:])

                for st4 in range(SC):  # v, natural layout
                    ps = pA.tile([P, HF], FP32, tag="pa")
                    for kc in range(KC):
                        nc.tensor.matmul(
                            ps[:], xb[:, kc, st4 * P:(st4 + 1) * P], wv_s[:, kc, :],
                            start=(kc == 0), stop=(kc == KC - 1),
                        )
                    nc.vector.tensor_copy(v_b[:, st4, :], ps[:])

                # ---- phase B: attention per head ----
                # Causal structure: kv chunk t8 >= SC only reaches queries
                # s >= (t8-SC)*P; the rest is masked, so narrow every op.
                aT = apool.tile([P, HPC, SEQ], BF16D)
                for h in range(HPC):
                    ee = epool.tile([P, TCH, SEQ], BF16D)
                    for t8 in range(TCH):
                        s0 = 0 if t8 < SC else (t8 - SC) * P
                        if t8 < SC:
                            klhs = kThb[:, h, t8 * P:(t8 + 1) * P]
                        else:
                            klhs = kT_b[:, h, (t8 - SC) * P:(t8 - SC + 1) * P]
                        pscr = pS.tile([P, SEQ], FP32, tag="ps")
                        nc.tensor.matmul(pscr[:, s0:], klhs, qT_b[:, h, s0:], start=True, stop=True)
                        tmpe = wkpool.tile([P, SEQ], BF16D, tag="te")
                        nc.scalar.activation(tmpe[:, s0:], pscr[:, s0:], mybir.ActivationFunctionType.Exp)
                        nc.vector.tensor_mul(ee[:, t8, s0:], tmpe[:, s0:], gtb[:, t8, s0:])
                    # row-sum over kv (partition axis) via ones-matmul
                    pr = pR.tile([1, SEQ], FP32, tag="pr")
                    for t8 in range(TCH):
                        s0 = 0 if t8 < SC else (t8 - SC) * P
                        nc.tensor.matmul(
                            pr[:, s0:], ones_bf[:], ee[:, t8, s0:],
                            start=(t8 == 0), stop=(t8 == TCH - 1),
                            skip_group_check=True,
                        )
                    rinv = smpool.tile([1, SEQ], FP32, tag="rinv")
                    nc.vector.reciprocal_approx_fast(rinv[:], pr[:])
                    rb_s = smpool.tile([P, SEQ], FP32, tag="rbs")
                    nc.gpsimd.partition_broadcast(rb_s[:], rinv[:])
                    # oT = V^T E (accumulate over kv chunks)
                    po = pOT.tile([P, SEQ], FP32, tag="po")
                    for t8 in range(TCH):
                        s0 = 0 if t8 < SC else (t8 - SC) * P
                        if t8 < SC:
                            vlhs = vhb[:, h * SC + t8, :]
                        else:
                            vlhs = v_b[:, t8 - SC, h * P:(h + 1) * P]
                        nc.tensor.matmul(
                            po[:, s0:], vlhs, ee[:, t8, s0:],
                            start=(t8 == 0), stop=(t8 == TCH - 1),
                            skip_group_check=True,
                        )
                    nc.vector.tensor_mul(aT[:, h, :], po[:], rb_s[:])

                # ---- o-projection (partial, row-parallel) ----
                for sc4 in range(SC):
                    for og in range(TCH // 2):
                        pout = popool.tile([P, 2, HF], BF16D, tag="pout")
                        for oi in range(2):
                            oc = og * 2 + oi
                            pp = pP.tile([P, HF], FP32, tag="pp")
                            for h in range(HPC):
                                nc.tensor.matmul(
                                    pp[:],
                                    aT[:, h, sc4 * P:(sc4 + 1) * P],
                                    woT_s[:, h, oc * HF:(oc + 1) * HF],
                                    start=(h == 0), stop=(h == HPC - 1),
                                )
                            nc.vector.tensor_copy(pout[:, oi, :], pp[:])
                        nc.sync.dma_start(
                            out_d[ts + sc4 * P: ts + (sc4 + 1) * P,
                                  og * 2 * HF:(og * 2 + 2) * HF],
                            pout[:],
                        )
    nc.finalize()
    return nc


_CACHE = {}


def _get_program():
    if "nc" not in _CACHE:
        _CACHE["nc"] = build_program()
    return _CACHE["nc"]


def _prep_inputs(inputs):
    x = np.asarray(inputs["x"], np.float32)
    router = np.asarray(inputs["router"], np.float32)
    cache_k = np.asarray(inputs["cache_k"], np.float32)
    cache_v = np.asarray(inputs["cache_v"], np.float32)
    cache_mask = np.asarray(inputs["cache_mask"])
    mask = np.asarray(inputs["mask"], np.float32)
    wq = np.asarray(inputs["wq"], np.float32)
    wk = np.asarray(inputs["wk"], np.float32)
    wv = np.asarray(inputs["wv"], np.float32)
    wo = np.asarray(inputs["wo"], np.float32)
    position_ids = np.asarray(inputs["position_ids"], np.int64)
    batch_exec = np.asarray(inputs["batch_exec"], np.int64)
    start_pos = int(inputs["start_pos"])
    assert start_pos == START and x.shape == (BSZ, SEQ, DIM)

    xT = np.ascontiguousarray(x.reshape(BSZ * SEQ, DIM).T).astype(BF16)

    # RoPE tables gathered at position_ids, in [hd, tok] layout
    inv_freq = 1.0 / (ROPE_BASE ** (np.arange(0, HD, 2, dtype=np.float32) / HD))
    t = np.arange(KV, dtype=np.float32)
    emb = np.concatenate([t[:, None] * inv_freq, t[:, None] * inv_freq], axis=-1)
    cos_t = np.cos(emb).astype(np.float32)[position_ids].reshape(BSZ * SEQ, HD)
    sin_t = np.sin(emb).astype(np.float32)[position_ids].reshape(BSZ * SEQ, HD)
    sign = np.where(np.arange(HD) < HD // 2, -1.0, 1.0).astype(np.float32)
    scale = np.float32(1.0 / np.sqrt(HD))
    rope = np.stack([
        cos_t.T * scale,                # cq (q, score-scale folded)
        (sin_t * sign).T * scale,       # sq signed
        cos_t.T,                        # ck
        (sin_t * sign).T,               # sk signed
    ]).astype(np.float32)               # [4, 128, 4096]

    # history cache slices (host-side gather = sharding)
    k_hist = cache_k[batch_exec, :, :START, :]   # [8, 32, 512, 128]
    v_hist = cache_v[batch_exec, :, :START, :]

    # multiplicative 0/1 gate, transposed [b, t, s]: causal AND cache-usable
    pen_hist = cache_mask[batch_exec, :START].astype(bool)            # [8, 512]
    pen_new = (router[:, :, 0] != 0.0)                                # [8, 512]
    pen = np.concatenate([pen_hist, pen_new], axis=1)                 # [8, 1024]
    causal_ok = (mask[0, 0] > -0.5)                                   # [512 s, 1024 t]
    gate = causal_ok.T[None, :, :] & pen[:, :, None]                  # [8, 1024, 512]
    gateT = np.ascontiguousarray(gate.astype(np.float32)).astype(BF16)

    in_maps = []
    for c in range(NC):
        hs, he = c * HPC, (c + 1) * HPC
        fs, fe = c * HF, (c + 1) * HF
        wqkvT = np.concatenate(
            [wq[fs:fe].T, wk[fs:fe].T, wv[fs:fe].T], axis=1).astype(BF16)
        woT = np.ascontiguousarray(wo[:, fs:fe].T).astype(BF16)
        kTh = np.ascontiguousarray(
            k_hist[:, hs:he].transpose(0, 1, 3, 2)).astype(BF16)
        vh = np.ascontiguousarray(v_hist[:, hs:he]).astype(BF16)
        in_maps.append({
            "xT": xT,
            "wqkvT": np.ascontiguousarray(wqkvT),
            "woT": woT,
            "rope": rope,
            "kTh": kTh,
            "vh": vh,
            "gateT": gateT,
        })
    return in_maps


def _install_profile_hook():
    """The agent image's antenv lacks axon_hooks; shim it so trace=True works."""
    import sys, types
    if "antenv.axon_hooks" in sys.modules:
        return
    try:
        from trn_agent_boot.trn_boot import _ntff_profile_via_ctypes
    except ImportError:
        return
    mod = types.ModuleType("antenv.axon_hooks")
    mod._hook = _ntff_profile_via_ctypes("/opt/axon/libaxon_pjrt.so")

    def set_axon_ntff_profile_hook(h):
        mod._hook = h

    def get_axon_ntff_profile_hook():
        return mod._hook

    mod.set_axon_ntff_profile_hook = set_axon_ntff_profile_hook
    mod.get_axon_ntff_profile_hook = get_axon_ntff_profile_hook
    sys.modules["antenv.axon_hooks"] = mod
    import antenv
    antenv.axon_hooks = mod


def _run(inputs, trace=False):
    if trace:
        _install_profile_hook()
    nc = _get_program()
    in_maps = _prep_inputs(inputs)
    res = run_bass_kernel_spmd(nc, in_maps, core_ids=list(range(NC)), trace=trace)
    acc = np.zeros((BSZ * SEQ, DIM), np.float32)
    for c in range(NC):
        acc += res.results[c]["out"].astype(np.float32)
    return acc.reshape(BSZ, SEQ, DIM), res


def kernel(**inputs):
    out, _ = _run(inputs, trace=False)
    return out
